# revision 36
# baseline (speedup 1.0000x reference)
"""Trainium2 Bass kernel for nn_NoSoftmaxGPT2Model (4-layer GPT2, no softmax).

Strategy: the missing softmax makes attention linear, so (Q K^T) V is
reassociated to Q (K^T V) -- K^T V is only [64, 64] per head. This kills the
S x S attention entirely and makes every op except that contraction
token-local. We shard the 2048-token sequence across 8 NeuronCores (256
tokens each), replicate the weights, and per layer AllReduce only the tiny
[12, 64, 64] K^T V partial sums (196 KB).

On-chip layout: activations live in SBUF transposed, [feature_part, token_free]
(T-layout). LayerNorm stats (per-token sums over features = partition
reduction) are computed with ones-vector matmuls on the PE, broadcast back
with a rank-1 ones matmul. LN gains are folded into the following weight
matrices on the host, biases are applied via ACT bias or rank-1 matmuls.

kernel(**inputs) takes the full unsharded inputs and returns the full
[1, 2048, 768] output.
"""

import os
from contextlib import ExitStack

import numpy as np
import ml_dtypes

import jax
from jax.sharding import Mesh, PartitionSpec, NamedSharding

import concourse.bass as bass
import concourse.bacc as bacc
import concourse.mybir as mybir
import concourse.tile as tile
from concourse.tile import add_dep_helper
from concourse import bass2jax
from concourse.masks import make_identity

from jax.experimental.shard_map import shard_map

N_CORES = 8
L, S, E, H, FF = 4, 2048, 768, 12, 3072
DH = E // H  # 64
T = S // N_CORES  # 256 tokens per core
KT = E // 128  # 6 feature tiles
FT = FF // 128  # 24 ff tiles
EPS = 1e-5

F32 = mybir.dt.float32
AF = mybir.ActivationFunctionType
AO = mybir.AluOpType

# "f32" | "bf16" | "f32r"
COMPUTE = os.environ.get("KERNEL_COMPUTE", "bf16")

# CoreSim doesn't implement Gelu; swap for Relu in timing-sim builds
GELU = AF.Relu if os.environ.get("KERNEL_SIM_GELU") else AF.Gelu

# fp8(e3m4) DoubleRow MLP: W1/W2 + h2/z in fp8, weights host-scaled by W_SC
# (descaled via the gelu `scale` and the residual-combine stt). ~1.44x PE on
# the two big matmuls + halved W1/W2 HBM traffic.
MLP_FP8 = os.environ.get("KERNEL_MLP_FP8", "1") == "1"
F8 = mybir.dt.float8e4  # DoubleRow requires e4m3/e5m2
W_SC = 64.0


def _dtw():
    return mybir.dt.bfloat16 if COMPUTE == "bf16" else mybir.dt.float32


def _mmv(ap):
    """View a matmul operand as float32r in f32r mode (1.33x PE throughput)."""
    if COMPUTE == "f32r":
        return ap.bitcast(mybir.dt.float32r)
    return ap


def build_model(reps=1, n_layers=L, collective=True):
    dtw = _dtw()
    nc = bacc.Bacc(
        "TRN2", target_bir_lowering=False, debug=False, num_devices=N_CORES
    )

    # xin: host-prepacked T-layout [128, 2*KT*T]: cols 0:KT*T = emb, KT*T: = wpe
    xin_d = nc.dram_tensor("xin", [128, 2 * KT * T], F32, kind="ExternalInput").ap()
    # weights arrive host-prepacked as [128, cols] SBUF images so each layer's
    # matrix is ONE large contiguous DMA (col block k = 128-row slice k)
    wq_d = nc.dram_tensor("wq", [L, 128, KT * E], dtw, kind="ExternalInput").ap()
    wkv_d = nc.dram_tensor("wkv", [L, 128, KT * 2 * E], dtw, kind="ExternalInput").ap()
    wo_d = nc.dram_tensor("wo", [L, 128, KT * E], dtw, kind="ExternalInput").ap()
    dt_mlp = F8 if MLP_FP8 else dtw
    w1_d = nc.dram_tensor("w1", [L, 4, 128, KT * E], dt_mlp, kind="ExternalInput").ap()
    w2_d = nc.dram_tensor("w2", [L, 4, 128, KT * E], dt_mlp, kind="ExternalInput").ap()
    # bias: host-prepacked f32 const block [128, 180]:
    #   cols 0:24 bq | 24:48 bo | 48:72 b2 | 72:168 b1 | 168:174 lnfg | 174:180 lnfb
    bias_d = nc.dram_tensor(
        "bias", [128, 3 * L * KT + L * FT + 2 * KT], F32, kind="ExternalInput"
    ).ap()
    bkv_d = nc.dram_tensor("bkv", [L, 2 * E], dtw, kind="ExternalInput").ap()
    out_d = nc.dram_tensor("out", [128, KT * T], F32, kind="ExternalOutput").ap()

    with tile.TileContext(nc) as tc, ExitStack() as ctx:
        const = ctx.enter_context(tc.tile_pool(name="const", bufs=1))
        wpool = ctx.enter_context(tc.tile_pool(name="wpool", bufs=1))
        apool = ctx.enter_context(tc.tile_pool(name="apool", bufs=1))
        ps = ctx.enter_context(tc.tile_pool(name="ps", bufs=1, space="PSUM"))
        dram = ctx.enter_context(tc.tile_pool(name="dram", bufs=1, space="DRAM"))

        _prev_dma = [None]
        _prev_adma = [None]

        def sdma(dst, src):
            """sync-queue DMA with forced emission-order enqueue (prevents
            scheduler-reordered slot-wait deadlocks in the shared FIFO)."""
            inst = nc.sync.dma_start(dst, src)
            if _prev_dma[0] is not None:
                add_dep_helper(inst.ins, _prev_dma[0].ins, sync=False, reason="dma order")
            _prev_dma[0] = inst
            return inst

        def adma(dst, src):
            """same, on the second HWDGE ring (ACT engine) -- used for the MLP
            weight stream so it doesn't serialize behind the attn weights."""
            inst = nc.scalar.dma_start(dst, src)
            if _prev_adma[0] is not None:
                add_dep_helper(inst.ins, _prev_adma[0].ins, sync=False, reason="dma order2")
            _prev_adma[0] = inst
            return inst

        # input first on the DMA FIFO so xT is ready earliest
        xall = const.tile([128, 2 * KT * T], F32, tag="xall")
        sdma(xall, xin_d)
        # packed bias block: one DMA
        NB = 3 * L * KT + L * FT + 2 * KT
        bias_sb = const.tile([128, NB], F32, tag="bias")
        sdma(bias_sb, bias_d)
        bq_all = bias_sb[:, 0 : L * KT]
        bo_all = bias_sb[:, L * KT : 2 * L * KT]
        b2_all = bias_sb[:, 2 * L * KT : 3 * L * KT]
        b1_all = bias_sb[:, 3 * L * KT : 3 * L * KT + L * FT]
        lnfg_sb = bias_sb[:, 3 * L * KT + L * FT : 3 * L * KT + L * FT + KT]
        lnfb_sb = bias_sb[:, 3 * L * KT + L * FT + KT : NB]
        # bkv rows at partitions 32*l (rank-1 bias matmul operands): one DMA
        bkv_all = const.tile([128, 2 * E], dtw, tag="bkv_all")
        sdma(bkv_all[0:128:32, :], bkv_d)
        ones_c = const.tile([128, 1], F32, tag="ones_c")
        nc.vector.memset(ones_c, 1.0)
        ones_r = const.tile([1, 128], F32, tag="ones_r")
        nc.vector.memset(ones_r, 1.0)
        eps_c = const.tile([1, 1], F32, tag="eps_c")
        nc.vector.memset(eps_c, EPS)
        ones32 = const.tile([128, 128], dtw, tag="ones32")
        nc.vector.memset(ones32, 1.0)

        def layernorm(x_tiles, out_dt, out_tag, out_bufs, gcol=None, bcol=None, outs=None):
            """(x - mu) * rsqrt(var + eps) per token; x in T-layout f32.

            Per-token (free-position) stats via ones-matmul partition
            reductions; broadcast [1,:] -> [128,:] via rank-1 ones matmul.
            Optional per-feature (partition) gain/bias applied via ACT.
            """
            stat = ps.tile([128, 512], F32, tag="pp", bufs=6, name="stat")[0:1, :]
            sq = []
            for k in range(KT):
                sqt = apool.tile([128, T], F32, tag="sq", bufs=2)
                nc.vector.tensor_mul(sqt, x_tiles[k], x_tiles[k])
                sq.append(sqt)
            for k in range(KT):
                nc.tensor.matmul(
                    stat[:, 0:T], ones_c, x_tiles[k], start=(k == 0), stop=(k == KT - 1)
                )
            for k in range(KT):
                nc.tensor.matmul(
                    stat[:, T : 2 * T], ones_c, sq[k], start=(k == 0), stop=(k == KT - 1)
                )
            mu2 = apool.tile([1, T], F32, tag="mu2", bufs=1)
            nc.scalar.activation(mu2, stat[:, 0:T], AF.Square, scale=1.0 / E)
            var = apool.tile([1, T], F32, tag="var", bufs=1)
            nc.vector.scalar_tensor_tensor(
                var, stat[:, T : 2 * T], 1.0 / E, mu2, op0=AO.mult, op1=AO.subtract
            )
            # rsmu: cols 0:T = rsqrt(var+eps), cols T:2T = mu * rs
            rsmu = apool.tile([1, 2 * T], F32, tag="rsmu", bufs=1)
            sd = apool.tile([1, T], F32, tag="sd", bufs=1)
            nc.scalar.activation(sd, var, AF.Sqrt, bias=eps_c)
            nc.vector.reciprocal(rsmu[:, 0:T], sd)
            nc.vector.scalar_tensor_tensor(
                rsmu[:, T : 2 * T],
                stat[:, 0:T],
                1.0 / E,
                rsmu[:, 0:T],
                op0=AO.mult,
                op1=AO.mult,
            )
            bc = ps.tile([128, 512], F32, tag="pp", bufs=6, name="bc")
            nc.tensor.matmul(bc, ones_r, rsmu, start=True, stop=True)
            res = []
            for k in range(KT):
                tmp = apool.tile([128, T], F32, tag="lntmp", bufs=2)
                nc.vector.tensor_mul(tmp, x_tiles[k], bc[:, 0:T])
                ot = (
                    outs[k]
                    if outs is not None
                    else apool.tile([128, T], out_dt, tag=out_tag, bufs=out_bufs)
                )
                if gcol is None:
                    nc.vector.tensor_sub(ot, tmp, bc[:, T : 2 * T])
                else:
                    tmp2 = apool.tile([128, T], F32, tag="lntmp2", bufs=2)
                    nc.vector.tensor_sub(tmp2, tmp, bc[:, T : 2 * T])
                    nc.vector.tensor_scalar(
                        ot,
                        tmp2,
                        gcol[:, k : k + 1],
                        bcol[:, k : k + 1],
                        op0=AO.mult,
                        op1=AO.add,
                    )
                res.append(ot)
            return res

        def layer(l, xT):
            # ---- per-layer bias views into const tables ----
            bq_sb = bq_all[:, l * KT : (l + 1) * KT]
            bo_sb = bo_all[:, l * KT : (l + 1) * KT]
            b2_sb = b2_all[:, l * KT : (l + 1) * KT]
            b1_sb = b1_all[:, l * FT : (l + 1) * FT]

            # ---- layer weight loads: one big DMA per matrix (prepacked) ----
            wkvt = wpool.tile([128, KT * 2 * E], dtw, tag="wkv", bufs=2)
            sdma(wkvt, wkv_d[l])
            wqt = wpool.tile([128, KT * E], dtw, tag="wq", bufs=2)
            sdma(wqt, wq_d[l])
            wot = wpool.tile([128, KT * E], dtw, tag="wo", bufs=2)
            sdma(wot, wo_d[l])
            wkv_sb = [wkvt[:, k * 2 * E : (k + 1) * 2 * E] for k in range(KT)]
            wq_sb = [wqt[:, k * E : (k + 1) * E] for k in range(KT)]
            wo_sb = [wot[:, k * E : (k + 1) * E] for k in range(KT)]

            # ---- LN1 ----
            hT = layernorm(xT, dtw, "hT", 7)
            if os.environ.get("KERNEL_STOP") == "A":
                return xT

            # ---- K,V: stationary = hT slices, moving = Wkv (N-layout out) ----
            kv_ps = [
                [
                    ps.tile([128, 512], F32, tag="pp", bufs=6, name=f"kv_ps_{m}_{n}")
                    for n in range(3)
                ]
                for m in range(2)
            ]
            for k in range(KT):
                for m in range(2):
                    for n in range(3):
                        nc.tensor.matmul(
                            kv_ps[m][n],
                            _mmv(hT[k][:, m * 128 : (m + 1) * 128]),
                            _mmv(wkv_sb[k][:, n * 512 : (n + 1) * 512]),
                            start=(k == 0),
                            stop=False,
                        )
            for m in range(2):
                for n in range(3):
                    # rank-1 bias add: ones(tokens) x bkv row (row 32*l)
                    nc.tensor.matmul(
                        kv_ps[m][n],
                        ones32[32 * l : 32 * l + 1, :],
                        bkv_all[32 * l : 32 * l + 1, n * 512 : (n + 1) * 512],
                        start=False,
                        stop=True,
                        tile_position=(32 * l, 0),
                    )
            KV = []
            for m in range(2):
                kvt = apool.tile([128, 2 * E], dtw, tag="KV", bufs=2)
                for n in range(3):
                    nc.vector.tensor_copy(
                        kvt[:, n * 512 : (n + 1) * 512], kv_ps[m][n]
                    )
                KV.append(kvt)

            if os.environ.get("KERNEL_STOP") == "B":
                return xT
            # ---- K^T V partials (contraction over local tokens) ----
            ktv_ps = ps.tile([128, 512], F32, tag="pp", bufs=6, name="ktv_ps")[:, 0:6*DH]
            for j in range(6):
                for i in range(2):
                    h = 2 * j + i
                    for m in range(2):
                        nc.tensor.matmul(
                            ktv_ps[i * 64 : (i + 1) * 64, j * 64 : (j + 1) * 64],
                            _mmv(KV[m][:, h * DH : (h + 1) * DH]),
                            _mmv(KV[m][:, E + h * DH : E + (h + 1) * DH]),
                            start=(m == 0),
                            stop=(m == 1),
                            tile_position=(0, i * 64),
                        )
            ktv_sb = apool.tile([128, 6 * DH], dtw, tag="ktv_sb", bufs=2)
            nc.vector.tensor_copy(ktv_sb, ktv_ps)

            if collective:
                cc_in = dram.tile([128, 6 * DH], dtw, tag="cc_in", bufs=2)
                cc_out = dram.tile(
                    [128, 6 * DH], dtw, tag="cc_out", bufs=2, addr_space="Shared"
                )
                nc.gpsimd.dma_start(cc_in, ktv_sb)
                nc.gpsimd.collective_compute(
                    "AllReduce",
                    AO.add,
                    ins=[cc_in.opt()],
                    outs=[cc_out.opt()],
                    replica_groups=[list(range(N_CORES))],
                )
                ktv_w = apool.tile([128, 6 * DH], dtw, tag="ktv_f", bufs=2)
                nc.gpsimd.dma_start(ktv_w, cc_out)
            else:
                ktv_w = ktv_sb

            if os.environ.get("KERNEL_STOP") == "D":
                return xT
            # ---- Q^T: stationary = Wq columns, moving = hT (T-layout out) ----
            QT = []
            for m in range(KT):
                qps = ps.tile([128, 512], F32, tag="pp", bufs=6, name="q_ps")[:, 0:T]
                for k in range(KT):
                    nc.tensor.matmul(
                        qps,
                        _mmv(wq_sb[k][:, m * 128 : (m + 1) * 128]),
                        _mmv(hT[k]),
                        start=(k == 0),
                        stop=(k == KT - 1),
                    )
                qt = apool.tile([128, T], dtw, tag="QT", bufs=7)
                nc.vector.tensor_scalar(
                    qt, qps, bq_sb[:, m : m + 1], None, op0=AO.add
                )
                QT.append(qt)

            if os.environ.get("KERNEL_STOP") == "C":
                return xT
            # ---- a^T = KtV^T-ish: lhsT = KtV[d1, d2] slice, rhs = Q^T head ----
            a_ps = [
                ps.tile([128, 512], F32, tag="pp", bufs=6, name=f"a_ps_{j}")[:, 0:T]
                for j in range(6)
            ]
            for j in range(6):
                for i in range(2):
                    h = 2 * j + i
                    nc.tensor.matmul(
                        a_ps[j][i * 64 : (i + 1) * 64, :],
                        _mmv(ktv_w[i * 64 : (i + 1) * 64, j * 64 : (j + 1) * 64]),
                        _mmv(QT[j][i * 64 : (i + 1) * 64, :]),
                        start=True,
                        stop=True,
                        tile_position=(i * 64, i * 64),
                    )
            aT = []
            for j in range(6):
                at = apool.tile([128, T], dtw, tag="aT", bufs=7)
                nc.scalar.activation(at, a_ps[j], AF.Copy)
                aT.append(at)

            if os.environ.get("KERNEL_STOP") == "E":
                return xT
            # ---- o = a @ Wo + bo + x (residual) ----
            x2T = []
            for m in range(KT):
                ops_ = ps.tile([128, 512], F32, tag="pp", bufs=6, name="o_ps")[:, 0:T]
                for k in range(KT):
                    nc.tensor.matmul(
                        ops_,
                        _mmv(wo_sb[k][:, m * 128 : (m + 1) * 128]),
                        _mmv(aT[k]),
                        start=(k == 0),
                        stop=(k == KT - 1),
                    )
                x2 = apool.tile([128, T], F32, tag="x2T", bufs=7)
                nc.vector.scalar_tensor_tensor(
                    x2, ops_, bo_sb[:, m : m + 1], xT[m], op0=AO.add, op1=AO.add
                )
                x2T.append(x2)

            if os.environ.get("KERNEL_STOP") == "F":
                return x2T
            # ---- LN2 ----
            if MLP_FP8:
                # h2 written as 3 fp8 pair-tiles [128, 2T] (k-even | k-odd halves)
                # so each DoubleRow matmul contracts two k-tiles at once
                h2pair = [
                    apool.tile([128, 2 * T], F8, tag="hT8", bufs=4, name=f"h2pair_{j}")
                    for j in range(3)
                ]
                houts = [h2pair[k // 2][:, (k % 2) * T : (k % 2 + 1) * T] for k in range(KT)]
                layernorm(x2T, None, None, None, outs=houts)
            else:
                h2T = layernorm(x2T, dtw, "hT", 7)
            if os.environ.get("KERNEL_STOP") == "G":
                return x2T

            # ---- fused MLP, ff-tile streaming: per 128-wide ff tile, accumulate
            # z_ps over the 6 input k-tiles, gelu into zc, then contract zc into
            # 6 PERSISTENT m_ps PSUM banks (held across all 24 ff tiles).
            # Residual + b2 are added once from PSUM at the very end -- no
            # intermediate SBUF accumulator round trips.
            m_ps = [
                ps.tile([128, 512], F32, tag="pp", bufs=6, name=f"m_ps_{m}")[:, 0:T]
                for m in range(KT)
            ]
            DR = mybir.MatmulPerfMode.DoubleRow
            zpair = None
            for cf in range(4):
                w1t = wpool.tile([128, KT * E], dt_mlp, tag="w1", bufs=2)
                adma(w1t, w1_d[l, cf])
                w2t = wpool.tile([128, KT * E], dt_mlp, tag="w2", bufs=2)
                adma(w2t, w2_d[l, cf])
                for fi in range(KT):
                    f = cf * KT + fi
                    z_ps = ps.tile([128, 512], F32, tag="zp", bufs=2, name=f"z_ps_{f}")[
                        :, 0:T
                    ]
                    if MLP_FP8:
                        for j in range(3):
                            base = (fi * 3 + j) * 2 * 128
                            nc.tensor.matmul(
                                z_ps,
                                w1t[:, base : base + 256].rearrange(
                                    "p (two m) -> p two m", two=2
                                ),
                                h2pair[j].rearrange("p (two n) -> p two n", two=2),
                                start=(j == 0),
                                stop=(j == 2),
                                perf_mode=DR,
                            )
                        jf, half = divmod(f, 2)
                        if half == 0:
                            zpair = apool.tile([128, 2 * T], F8, tag="zT", bufs=3)
                        nc.scalar.activation(
                            zpair[:, half * T : (half + 1) * T],
                            z_ps,
                            GELU,
                            bias=b1_sb[:, f : f + 1],
                            scale=1.0 / W_SC,
                        )
                        if half == 1:
                            jl = jf - cf * 3
                            for m in range(KT):
                                base = (jl * KT + m) * 2 * 128
                                nc.tensor.matmul(
                                    m_ps[m],
                                    w2t[:, base : base + 256].rearrange(
                                        "p (two m) -> p two m", two=2
                                    ),
                                    zpair.rearrange("p (two n) -> p two n", two=2),
                                    start=(jf == 0),
                                    stop=(jf == 2 * KT - 1),
                                    perf_mode=DR,
                                )
                    else:
                        for k in range(KT):
                            nc.tensor.matmul(
                                z_ps,
                                _mmv(w1t[:, (fi * KT + k) * 128 : (fi * KT + k + 1) * 128]),
                                _mmv(h2T[k]),
                                start=(k == 0),
                                stop=(k == KT - 1),
                            )
                        zt = apool.tile([128, T], dtw, tag="zT", bufs=4)
                        nc.scalar.activation(zt, z_ps, GELU, bias=b1_sb[:, f : f + 1])
                        for m in range(KT):
                            nc.tensor.matmul(
                                m_ps[m],
                                _mmv(w2t[:, (fi * KT + m) * 128 : (fi * KT + m + 1) * 128]),
                                _mmv(zt),
                                start=(f == 0),
                                stop=(f == 4 * KT - 1),
                            )
            nxt = []
            for m in range(KT):
                xn = apool.tile([128, T], F32, tag="xT", bufs=8)
                if MLP_FP8:
                    t = apool.tile([128, T], F32, tag="mres", bufs=2)
                    nc.vector.scalar_tensor_tensor(
                        t, m_ps[m], 1.0 / W_SC, x2T[m], op0=AO.mult, op1=AO.add
                    )
                    nc.vector.tensor_scalar(
                        xn, t, b2_sb[:, m : m + 1], None, op0=AO.add
                    )
                else:
                    nc.vector.scalar_tensor_tensor(
                        xn, m_ps[m], b2_sb[:, m : m + 1], x2T[m], op0=AO.add, op1=AO.add
                    )
                nxt.append(xn)
            return nxt

        for _rep in range(reps):
            # ---- x = emb + wpe, already in T-layout from the host ----
            xT = []
            for k in range(KT):
                xt = apool.tile([128, T], F32, tag="xT", bufs=8)
                nc.vector.tensor_add(
                    xt,
                    xall[:, k * T : (k + 1) * T],
                    xall[:, KT * T + k * T : KT * T + (k + 1) * T],
                )
                xT.append(xt)

            for l in range(n_layers):
                xT = layer(l, xT)

            # ---- final LN (with gain/bias), stored in T-layout ----
            fout = apool.tile([128, KT * T], F32, tag="fout", bufs=1)
            layernorm(
                xT,
                F32,
                None,
                None,
                gcol=lnfg_sb,
                bcol=lnfb_sb,
                outs=[fout[:, k * T : (k + 1) * T] for k in range(KT)],
            )
            sdma(out_d, fout)

    nc.compile()
    return nc


class SpmdRunner:
    """Reusable jitted SPMD runner (modeled on bass2jax.run_bass_via_pjrt,
    without donation, so it can be invoked repeatedly)."""

    def __init__(self, nc, n_cores=N_CORES):
        bass2jax.install_neuronx_cc_hook()
        self.nc = nc
        self.n_cores = n_cores
        partition_name = nc.partition_id_tensor.name if nc.partition_id_tensor else None
        in_names, out_names, out_avals = [], [], []
        for alloc in nc.m.functions[0].allocations:
            if not isinstance(alloc, mybir.MemoryLocationSet):
                continue
            name = alloc.memorylocations[0].name
            if alloc.kind == "ExternalInput":
                if name != partition_name:
                    in_names.append(name)
            elif alloc.kind == "ExternalOutput":
                out_names.append(name)
                out_avals.append(
                    jax.core.ShapedArray(
                        tuple(alloc.tensor_shape), mybir.dt.np(alloc.dtype)
                    )
                )
        self.in_names, self.out_names, self.out_avals = in_names, out_names, out_avals
        n_params = len(in_names)
        all_in_names = list(in_names) + list(out_names)
        if partition_name is not None:
            all_in_names.append(partition_name)

        def _body(*args):
            operands = list(args)
            if partition_name is not None:
                operands.append(bass2jax.partition_id_tensor())
            outs = bass2jax._bass_exec_p.bind(
                *operands,
                out_avals=tuple(out_avals),
                in_names=tuple(all_in_names),
                out_names=tuple(out_names),
                lowering_input_output_aliases=(),
                sim_require_finite=True,
                sim_require_nnan=True,
                nc=nc,
            )
            return tuple(outs)

        devices = jax.devices()[:n_cores]
        self.mesh = Mesh(np.asarray(devices), ("core",))
        n_outs = len(out_names)
        in_specs = (PartitionSpec("core"),) * (n_params + n_outs)
        out_specs = (PartitionSpec("core"),) * n_outs
        self.fn = jax.jit(
            shard_map(
                _body,
                mesh=self.mesh,
                in_specs=in_specs,
                out_specs=out_specs,
                check_rep=False,
            ),
            keep_unused=True,
        )
        self.args = None

    def stage(self, in_maps):
        n = self.n_cores
        concat_in = [
            np.concatenate([np.asarray(in_maps[c][name]) for c in range(n)], axis=0)
            for name in self.in_names
        ]
        concat_zero = [
            np.zeros((n * a.shape[0], *a.shape[1:]), a.dtype) for a in self.out_avals
        ]
        sh = NamedSharding(self.mesh, PartitionSpec("core"))
        self.args = [jax.device_put(a, sh) for a in concat_in + concat_zero]

    def run(self):
        return self.fn(*self.args)

    def results(self, out_arrs):
        n = self.n_cores
        return [
            {
                name: np.asarray(out_arrs[i]).reshape(n, *self.out_avals[i].shape)[c]
                for i, name in enumerate(self.out_names)
            }
            for c in range(n)
        ]


def preprocess(inputs):
    """Host-side: fold LN gains into weights, shard tokens, build in_maps."""
    f = np.float32
    ie = np.asarray(inputs["inputs_embeds"], f)[0]  # [S, E]
    wpe = np.asarray(inputs["wpe"], f)[:S]
    g1 = np.asarray(inputs["ln1_g"], f)
    b1l = np.asarray(inputs["ln1_b"], f)
    g2 = np.asarray(inputs["ln2_g"], f)
    b2l = np.asarray(inputs["ln2_b"], f)
    Wq = np.asarray(inputs["Wq"], f)
    Wk = np.asarray(inputs["Wk"], f)
    Wv = np.asarray(inputs["Wv"], f)
    Wo = np.asarray(inputs["Wo"], f)
    W1 = np.asarray(inputs["W1"], f)
    W2 = np.asarray(inputs["W2"], f)
    bq = np.asarray(inputs["bq"], f)
    bk = np.asarray(inputs["bk"], f)
    bv = np.asarray(inputs["bv"], f)
    bo = np.asarray(inputs["bo"], f)
    b1 = np.asarray(inputs["b1"], f)
    b2 = np.asarray(inputs["b2"], f)

    scale = 1.0 / np.sqrt(DH)
    Wq_p = g1[:, :, None] * Wq * scale
    bq_p = (np.einsum("le,lef->lf", b1l, Wq) + bq) * scale
    Wk_p = g1[:, :, None] * Wk
    bk_p = np.einsum("le,lef->lf", b1l, Wk) + bk
    Wv_p = g1[:, :, None] * Wv
    bv_p = np.einsum("le,lef->lf", b1l, Wv) + bv
    Wkv = np.concatenate([Wk_p, Wv_p], axis=2)
    bkv = np.concatenate([bk_p, bv_p], axis=1)
    W1_p = g2[:, :, None] * W1
    b1_p = np.einsum("le,lef->lf", b2l, W1) + b1

    if COMPUTE == "bf16":
        cast = lambda a: np.ascontiguousarray(a).astype(ml_dtypes.bfloat16)
    else:
        cast = lambda a: np.ascontiguousarray(a, f)

    # prepack to [128, cols] SBUF images: col block k = rows k*128:(k+1)*128
    def pack2(a):  # [L, R, C] -> [L, 128, (R/128)*C]
        Lr, R, C = a.shape
        return (
            a.reshape(Lr, R // 128, 128, C)
            .transpose(0, 2, 1, 3)
            .reshape(Lr, 128, (R // 128) * C)
        )

    if MLP_FP8:
        f8 = ml_dtypes.float8_e4m3
        # chunk cf: block (fi, j) = two k-planes [k=2j | k=2j+1] of W1 cols f*128
        W1_pk = (
            (W1_p * W_SC)
            .reshape(L, 3, 2, 128, 4, KT, 128)
            .transpose(0, 4, 3, 5, 1, 2, 6)
            .reshape(L, 4, 128, KT * E)
            .astype(f8)
        )
        # chunk cf: block (jf, m) = two ff-planes [fi=2jf | fi=2jf+1] of W2 cols m*128
        W2_pk = (
            (W2 * W_SC)
            .reshape(L, 4, 3, 2, 128, KT, 128)
            .transpose(0, 1, 4, 2, 5, 3, 6)
            .reshape(L, 4, 128, KT * E)
            .astype(f8)
        )
    else:
        # chunk cf holds ff-tiles f=cf*6+fi; block (fi,k) at cols (fi*6+k)*128
        W1_pk = cast(
            W1_p.reshape(L, KT, 128, 4, KT, 128)
            .transpose(0, 3, 2, 4, 1, 5)
            .reshape(L, 4, 128, KT * E)
        )
        # chunk cf: block (fi,m) at cols (fi*6+m)*128 = W2[(cf*6+fi)*128.., m*128..]
        W2_pk = cast(
            W2.reshape(L, 4, KT, 128, KT, 128)
            .transpose(0, 1, 3, 2, 4, 5)
            .reshape(L, 4, 128, KT * E)
        )

    # packed bias const block [128, 180]
    def bpack(a, n):  # [L, n*128] -> [128, L*n]
        return a.reshape(L, n, 128).transpose(2, 0, 1).reshape(128, L * n)

    bias_blk = np.concatenate(
        [
            bpack(bq_p, KT),
            bpack(bo, KT),
            bpack(b2, KT),
            bpack(b1_p, FT),
            np.asarray(inputs["lnf_g"], f).reshape(KT, 128).T,
            np.asarray(inputs["lnf_b"], f).reshape(KT, 128).T,
        ],
        axis=1,
    )

    def tpack(a):  # [T, E] -> [128, KT*T] T-layout
        return a.reshape(T, KT, 128).transpose(2, 1, 0).reshape(128, KT * T)

    common = {
        "wq": cast(pack2(Wq_p)),
        "wkv": cast(pack2(Wkv)),
        "wo": cast(pack2(Wo)),
        "w1": np.ascontiguousarray(W1_pk) if MLP_FP8 else W1_pk,
        "w2": np.ascontiguousarray(W2_pk) if MLP_FP8 else W2_pk,
        "bias": np.ascontiguousarray(bias_blk, f),
        "bkv": cast(bkv),
    }
    maps = []
    for c in range(N_CORES):
        sl = slice(c * T, (c + 1) * T)
        xin = np.concatenate([tpack(ie[sl]), tpack(wpe[sl])], axis=1)
        maps.append({**common, "xin": np.ascontiguousarray(xin, f)})
    return maps


_RUNNER = None


def _get_runner():
    global _RUNNER
    if _RUNNER is None:
        nc = build_model(reps=1)
        _RUNNER = SpmdRunner(nc)
    return _RUNNER


def kernel(**inputs):
    runner = _get_runner()
    maps = preprocess(inputs)
    runner.stage(maps)
    outs = runner.run()
    res = runner.results(outs)
    full = np.concatenate(
        [
            res[c]["out"].reshape(128, KT, T).transpose(2, 1, 0).reshape(T, E)
            for c in range(N_CORES)
        ],
        axis=0,
    )
    return full[None].astype(np.float32)



# revision 38
# speedup vs baseline: 1.0677x; 1.0677x over previous
"""Trainium2 Bass kernel for nn_NoSoftmaxGPT2Model (4-layer GPT2, no softmax).

Strategy: the missing softmax makes attention linear, so (Q K^T) V is
reassociated to Q (K^T V) -- K^T V is only [64, 64] per head. This kills the
S x S attention entirely and makes every op except that contraction
token-local. We shard the 2048-token sequence across 8 NeuronCores (256
tokens each), replicate the weights, and per layer AllReduce only the tiny
[12, 64, 64] K^T V partial sums (196 KB).

On-chip layout: activations live in SBUF transposed, [feature_part, token_free]
(T-layout). LayerNorm stats (per-token sums over features = partition
reduction) are computed with ones-vector matmuls on the PE, broadcast back
with a rank-1 ones matmul. LN gains are folded into the following weight
matrices on the host, biases are applied via ACT bias or rank-1 matmuls.

kernel(**inputs) takes the full unsharded inputs and returns the full
[1, 2048, 768] output.
"""

import os
from contextlib import ExitStack

import numpy as np
import ml_dtypes

import jax
from jax.sharding import Mesh, PartitionSpec, NamedSharding

import concourse.bass as bass
import concourse.bacc as bacc
import concourse.mybir as mybir
import concourse.tile as tile
from concourse.tile import add_dep_helper
from concourse import bass2jax
from concourse.masks import make_identity

from jax.experimental.shard_map import shard_map

N_CORES = 8
L, S, E, H, FF = 4, 2048, 768, 12, 3072
DH = E // H  # 64
T = S // N_CORES  # 256 tokens per core
KT = E // 128  # 6 feature tiles
FT = FF // 128  # 24 ff tiles
EPS = 1e-5

F32 = mybir.dt.float32
AF = mybir.ActivationFunctionType
AO = mybir.AluOpType

# "f32" | "bf16" | "f32r"
COMPUTE = os.environ.get("KERNEL_COMPUTE", "bf16")

# CoreSim doesn't implement Gelu; swap for Relu in timing-sim builds
GELU = AF.Relu if os.environ.get("KERNEL_SIM_GELU") else AF.Gelu

# fp8(e3m4) DoubleRow MLP: W1/W2 + h2/z in fp8, weights host-scaled by W_SC
# (descaled via the gelu `scale` and the residual-combine stt). ~1.44x PE on
# the two big matmuls + halved W1/W2 HBM traffic.
MLP_FP8 = os.environ.get("KERNEL_MLP_FP8", "1") == "1"
# bf16 residual stream: 4x faster LN-stat matmuls (f32 moving = 1/4 PE rate)
# and 2x DVE throughput on the x ops, at ~0.4%-per-add rounding cost
X_BF16 = os.environ.get("KERNEL_X_BF16", "1") == "1"
F8 = mybir.dt.float8e4  # DoubleRow requires e4m3/e5m2
W_SC = 64.0


def _dtw():
    return mybir.dt.bfloat16 if COMPUTE == "bf16" else mybir.dt.float32


def _mmv(ap):
    """View a matmul operand as float32r in f32r mode (1.33x PE throughput)."""
    if COMPUTE == "f32r":
        return ap.bitcast(mybir.dt.float32r)
    return ap


def build_model(reps=1, n_layers=L, collective=True):
    dtw = _dtw()
    xdt = dtw if X_BF16 else F32
    nc = bacc.Bacc(
        "TRN2", target_bir_lowering=False, debug=False, num_devices=N_CORES
    )

    # xin: host-prepacked T-layout [128, 2*KT*T]: cols 0:KT*T = emb, KT*T: = wpe
    xin_d = nc.dram_tensor("xin", [128, 2 * KT * T], F32, kind="ExternalInput").ap()
    # weights arrive host-prepacked as [128, cols] SBUF images so each layer's
    # matrix is ONE large contiguous DMA (col block k = 128-row slice k)
    wq_d = nc.dram_tensor("wq", [L, 128, KT * E], dtw, kind="ExternalInput").ap()
    wkv_d = nc.dram_tensor("wkv", [L, 128, KT * 2 * E], dtw, kind="ExternalInput").ap()
    wo_d = nc.dram_tensor("wo", [L, 128, KT * E], dtw, kind="ExternalInput").ap()
    dt_mlp = F8 if MLP_FP8 else dtw
    w1_d = nc.dram_tensor("w1", [L, 4, 128, KT * E], dt_mlp, kind="ExternalInput").ap()
    w2_d = nc.dram_tensor("w2", [L, 4, 128, KT * E], dt_mlp, kind="ExternalInput").ap()
    # bias: host-prepacked f32 const block [128, 180]:
    #   cols 0:24 bq | 24:48 bo | 48:72 b2 | 72:168 b1 | 168:174 lnfg | 174:180 lnfb
    bias_d = nc.dram_tensor(
        "bias", [128, 3 * L * KT + L * FT + 2 * KT], F32, kind="ExternalInput"
    ).ap()
    bkv_d = nc.dram_tensor("bkv", [L, 2 * E], dtw, kind="ExternalInput").ap()
    out_d = nc.dram_tensor("out", [128, KT * T], F32, kind="ExternalOutput").ap()

    with tile.TileContext(nc) as tc, ExitStack() as ctx:
        const = ctx.enter_context(tc.tile_pool(name="const", bufs=1))
        wpool = ctx.enter_context(tc.tile_pool(name="wpool", bufs=1))
        apool = ctx.enter_context(tc.tile_pool(name="apool", bufs=1))
        ps = ctx.enter_context(tc.tile_pool(name="ps", bufs=1, space="PSUM"))
        dram = ctx.enter_context(tc.tile_pool(name="dram", bufs=1, space="DRAM"))

        _prev_dma = [None]
        _prev_adma = [None]

        def sdma(dst, src):
            """sync-queue DMA with forced emission-order enqueue (prevents
            scheduler-reordered slot-wait deadlocks in the shared FIFO)."""
            inst = nc.sync.dma_start(dst, src)
            if _prev_dma[0] is not None:
                add_dep_helper(inst.ins, _prev_dma[0].ins, sync=False, reason="dma order")
            _prev_dma[0] = inst
            return inst

        def adma(dst, src):
            """same, on the second HWDGE ring (ACT engine) -- used for the MLP
            weight stream so it doesn't serialize behind the attn weights."""
            inst = nc.scalar.dma_start(dst, src)
            if _prev_adma[0] is not None:
                add_dep_helper(inst.ins, _prev_adma[0].ins, sync=False, reason="dma order2")
            _prev_adma[0] = inst
            return inst

        # input first on the DMA FIFO so xT is ready earliest
        xall = const.tile([128, 2 * KT * T], F32, tag="xall")
        sdma(xall, xin_d)
        # packed bias block: one DMA
        NB = 3 * L * KT + L * FT + 2 * KT
        bias_sb = const.tile([128, NB], F32, tag="bias")
        sdma(bias_sb, bias_d)
        bq_all = bias_sb[:, 0 : L * KT]
        bo_all = bias_sb[:, L * KT : 2 * L * KT]
        b2_all = bias_sb[:, 2 * L * KT : 3 * L * KT]
        b1_all = bias_sb[:, 3 * L * KT : 3 * L * KT + L * FT]
        lnfg_sb = bias_sb[:, 3 * L * KT + L * FT : 3 * L * KT + L * FT + KT]
        lnfb_sb = bias_sb[:, 3 * L * KT + L * FT + KT : NB]
        # bkv rows at partitions 32*l (rank-1 bias matmul operands): one DMA
        bkv_all = const.tile([128, 2 * E], dtw, tag="bkv_all")
        sdma(bkv_all[0:128:32, :], bkv_d)
        ones_c = const.tile([128, 1], xdt, tag="ones_c")
        nc.vector.memset(ones_c, 1.0)
        ones_r = const.tile([1, 128], F32, tag="ones_r")
        nc.vector.memset(ones_r, 1.0)
        eps_c = const.tile([1, 1], F32, tag="eps_c")
        nc.vector.memset(eps_c, EPS)
        ones32 = const.tile([128, 128], dtw, tag="ones32")
        nc.vector.memset(ones32, 1.0)

        def layernorm(x_tiles, out_dt, out_tag, out_bufs, gcol=None, bcol=None, outs=None):
            """(x - mu) * rsqrt(var + eps) per token; x in T-layout f32.

            Per-token (free-position) stats via ones-matmul partition
            reductions; broadcast [1,:] -> [128,:] via rank-1 ones matmul.
            Optional per-feature (partition) gain/bias applied via ACT.
            """
            stat = ps.tile([128, 512], F32, tag="pp", bufs=6, name="stat")[0:1, :]
            sq = []
            for k in range(KT):
                sqt = apool.tile([128, T], xdt, tag="sq", bufs=2)
                nc.vector.tensor_mul(sqt, x_tiles[k], x_tiles[k])
                sq.append(sqt)
            for k in range(KT):
                nc.tensor.matmul(
                    stat[:, 0:T], ones_c, x_tiles[k], start=(k == 0), stop=(k == KT - 1)
                )
            for k in range(KT):
                nc.tensor.matmul(
                    stat[:, T : 2 * T], ones_c, sq[k], start=(k == 0), stop=(k == KT - 1)
                )
            mu2 = apool.tile([1, T], F32, tag="mu2", bufs=1)
            nc.scalar.activation(mu2, stat[:, 0:T], AF.Square, scale=1.0 / E)
            var = apool.tile([1, T], F32, tag="var", bufs=1)
            nc.vector.scalar_tensor_tensor(
                var, stat[:, T : 2 * T], 1.0 / E, mu2, op0=AO.mult, op1=AO.subtract
            )
            # rsmu: cols 0:T = rsqrt(var+eps), cols T:2T = mu * rs
            rsmu = apool.tile([1, 2 * T], F32, tag="rsmu", bufs=1)
            sd = apool.tile([1, T], F32, tag="sd", bufs=1)
            nc.scalar.activation(sd, var, AF.Sqrt, bias=eps_c)
            nc.vector.reciprocal(rsmu[:, 0:T], sd)
            nc.vector.scalar_tensor_tensor(
                rsmu[:, T : 2 * T],
                stat[:, 0:T],
                1.0 / E,
                rsmu[:, 0:T],
                op0=AO.mult,
                op1=AO.mult,
            )
            bc = ps.tile([128, 512], F32, tag="pp", bufs=6, name="bc")
            nc.tensor.matmul(bc, ones_r, rsmu, start=True, stop=True)
            res = []
            for k in range(KT):
                tmp = apool.tile([128, T], F32, tag="lntmp", bufs=2)
                nc.vector.tensor_mul(tmp, x_tiles[k], bc[:, 0:T])
                ot = (
                    outs[k]
                    if outs is not None
                    else apool.tile([128, T], out_dt, tag=out_tag, bufs=out_bufs)
                )
                if gcol is None:
                    nc.vector.tensor_sub(ot, tmp, bc[:, T : 2 * T])
                else:
                    tmp2 = apool.tile([128, T], F32, tag="lntmp2", bufs=2)
                    nc.vector.tensor_sub(tmp2, tmp, bc[:, T : 2 * T])
                    nc.vector.tensor_scalar(
                        ot,
                        tmp2,
                        gcol[:, k : k + 1],
                        bcol[:, k : k + 1],
                        op0=AO.mult,
                        op1=AO.add,
                    )
                res.append(ot)
            return res

        def layer(l, xT):
            # ---- per-layer bias views into const tables ----
            bq_sb = bq_all[:, l * KT : (l + 1) * KT]
            bo_sb = bo_all[:, l * KT : (l + 1) * KT]
            b2_sb = b2_all[:, l * KT : (l + 1) * KT]
            b1_sb = b1_all[:, l * FT : (l + 1) * FT]

            # ---- layer weight loads: one big DMA per matrix (prepacked) ----
            wkvt = wpool.tile([128, KT * 2 * E], dtw, tag="wkv", bufs=2)
            sdma(wkvt, wkv_d[l])
            wqt = wpool.tile([128, KT * E], dtw, tag="wq", bufs=2)
            sdma(wqt, wq_d[l])
            wot = wpool.tile([128, KT * E], dtw, tag="wo", bufs=2)
            sdma(wot, wo_d[l])
            wkv_sb = [wkvt[:, k * 2 * E : (k + 1) * 2 * E] for k in range(KT)]
            wq_sb = [wqt[:, k * E : (k + 1) * E] for k in range(KT)]
            wo_sb = [wot[:, k * E : (k + 1) * E] for k in range(KT)]

            # ---- LN1 ----
            hT = layernorm(xT, dtw, "hT", 7)
            if os.environ.get("KERNEL_STOP") == "A":
                return xT

            # ---- K,V: stationary = hT slices, moving = Wkv (N-layout out) ----
            kv_ps = [
                [
                    ps.tile([128, 512], F32, tag="pp", bufs=6, name=f"kv_ps_{m}_{n}")
                    for n in range(3)
                ]
                for m in range(2)
            ]
            for k in range(KT):
                for m in range(2):
                    for n in range(3):
                        nc.tensor.matmul(
                            kv_ps[m][n],
                            _mmv(hT[k][:, m * 128 : (m + 1) * 128]),
                            _mmv(wkv_sb[k][:, n * 512 : (n + 1) * 512]),
                            start=(k == 0),
                            stop=False,
                        )
            for m in range(2):
                for n in range(3):
                    # rank-1 bias add: ones(tokens) x bkv row (row 32*l)
                    nc.tensor.matmul(
                        kv_ps[m][n],
                        ones32[32 * l : 32 * l + 1, :],
                        bkv_all[32 * l : 32 * l + 1, n * 512 : (n + 1) * 512],
                        start=False,
                        stop=True,
                        tile_position=(32 * l, 0),
                    )
            KV = []
            for m in range(2):
                kvt = apool.tile([128, 2 * E], dtw, tag="KV", bufs=2)
                for n in range(3):
                    nc.vector.tensor_copy(
                        kvt[:, n * 512 : (n + 1) * 512], kv_ps[m][n]
                    )
                KV.append(kvt)

            if os.environ.get("KERNEL_STOP") == "B":
                return xT
            # ---- K^T V partials (contraction over local tokens) ----
            ktv_ps = ps.tile([128, 512], F32, tag="pp", bufs=6, name="ktv_ps")[:, 0:6*DH]
            for j in range(6):
                for i in range(2):
                    h = 2 * j + i
                    for m in range(2):
                        nc.tensor.matmul(
                            ktv_ps[i * 64 : (i + 1) * 64, j * 64 : (j + 1) * 64],
                            _mmv(KV[m][:, h * DH : (h + 1) * DH]),
                            _mmv(KV[m][:, E + h * DH : E + (h + 1) * DH]),
                            start=(m == 0),
                            stop=(m == 1),
                            tile_position=(0, i * 64),
                        )
            ktv_sb = apool.tile([128, 6 * DH], dtw, tag="ktv_sb", bufs=2)
            nc.vector.tensor_copy(ktv_sb, ktv_ps)

            if collective:
                cc_in = dram.tile([128, 6 * DH], dtw, tag="cc_in", bufs=2)
                cc_out = dram.tile(
                    [128, 6 * DH], dtw, tag="cc_out", bufs=2, addr_space="Shared"
                )
                nc.gpsimd.dma_start(cc_in, ktv_sb)
                nc.gpsimd.collective_compute(
                    "AllReduce",
                    AO.add,
                    ins=[cc_in.opt()],
                    outs=[cc_out.opt()],
                    replica_groups=[list(range(N_CORES))],
                )
                ktv_w = apool.tile([128, 6 * DH], dtw, tag="ktv_f", bufs=2)
                nc.gpsimd.dma_start(ktv_w, cc_out)
            else:
                ktv_w = ktv_sb

            if os.environ.get("KERNEL_STOP") == "D":
                return xT
            # ---- Q^T: stationary = Wq columns, moving = hT (T-layout out) ----
            QT = []
            for m in range(KT):
                qps = ps.tile([128, 512], F32, tag="pp", bufs=6, name="q_ps")[:, 0:T]
                for k in range(KT):
                    nc.tensor.matmul(
                        qps,
                        _mmv(wq_sb[k][:, m * 128 : (m + 1) * 128]),
                        _mmv(hT[k]),
                        start=(k == 0),
                        stop=(k == KT - 1),
                    )
                qt = apool.tile([128, T], dtw, tag="QT", bufs=7)
                nc.vector.tensor_scalar(
                    qt, qps, bq_sb[:, m : m + 1], None, op0=AO.add
                )
                QT.append(qt)

            if os.environ.get("KERNEL_STOP") == "C":
                return xT
            # ---- a^T = KtV^T-ish: lhsT = KtV[d1, d2] slice, rhs = Q^T head ----
            a_ps = [
                ps.tile([128, 512], F32, tag="pp", bufs=6, name=f"a_ps_{j}")[:, 0:T]
                for j in range(6)
            ]
            for j in range(6):
                for i in range(2):
                    h = 2 * j + i
                    nc.tensor.matmul(
                        a_ps[j][i * 64 : (i + 1) * 64, :],
                        _mmv(ktv_w[i * 64 : (i + 1) * 64, j * 64 : (j + 1) * 64]),
                        _mmv(QT[j][i * 64 : (i + 1) * 64, :]),
                        start=True,
                        stop=True,
                        tile_position=(i * 64, i * 64),
                    )
            aT = []
            for j in range(6):
                at = apool.tile([128, T], dtw, tag="aT", bufs=7)
                nc.scalar.activation(at, a_ps[j], AF.Copy)
                aT.append(at)

            if os.environ.get("KERNEL_STOP") == "E":
                return xT
            # ---- o = a @ Wo + bo + x (residual) ----
            x2T = []
            for m in range(KT):
                ops_ = ps.tile([128, 512], F32, tag="pp", bufs=6, name="o_ps")[:, 0:T]
                for k in range(KT):
                    nc.tensor.matmul(
                        ops_,
                        _mmv(wo_sb[k][:, m * 128 : (m + 1) * 128]),
                        _mmv(aT[k]),
                        start=(k == 0),
                        stop=(k == KT - 1),
                    )
                x2 = apool.tile([128, T], xdt, tag="x2T", bufs=7)
                nc.vector.scalar_tensor_tensor(
                    x2, ops_, bo_sb[:, m : m + 1], xT[m], op0=AO.add, op1=AO.add
                )
                x2T.append(x2)

            if os.environ.get("KERNEL_STOP") == "F":
                return x2T
            # ---- LN2 ----
            if MLP_FP8:
                # h2 written as 3 fp8 pair-tiles [128, 2T] (k-even | k-odd halves)
                # so each DoubleRow matmul contracts two k-tiles at once
                h2pair = [
                    apool.tile([128, 2 * T], F8, tag="hT8", bufs=4, name=f"h2pair_{j}")
                    for j in range(3)
                ]
                houts = [h2pair[k // 2][:, (k % 2) * T : (k % 2 + 1) * T] for k in range(KT)]
                layernorm(x2T, None, None, None, outs=houts)
            else:
                h2T = layernorm(x2T, dtw, "hT", 7)
            if os.environ.get("KERNEL_STOP") == "G":
                return x2T

            # ---- fused MLP, ff-tile streaming: per 128-wide ff tile, accumulate
            # z_ps over the 6 input k-tiles, gelu into zc, then contract zc into
            # 6 PERSISTENT m_ps PSUM banks (held across all 24 ff tiles).
            # Residual + b2 are added once from PSUM at the very end -- no
            # intermediate SBUF accumulator round trips.
            m_ps = [
                ps.tile([128, 512], F32, tag="pp", bufs=6, name=f"m_ps_{m}")[:, 0:T]
                for m in range(KT)
            ]
            DR = mybir.MatmulPerfMode.DoubleRow
            zpair = None
            for cf in range(4):
                w1t = wpool.tile([128, KT * E], dt_mlp, tag="w1", bufs=2)
                adma(w1t, w1_d[l, cf])
                w2t = wpool.tile([128, KT * E], dt_mlp, tag="w2", bufs=2)
                adma(w2t, w2_d[l, cf])
                for fi in range(KT):
                    f = cf * KT + fi
                    z_ps = ps.tile([128, 512], F32, tag="zp", bufs=2, name=f"z_ps_{f}")[
                        :, 0:T
                    ]
                    if MLP_FP8:
                        for j in range(3):
                            base = (fi * 3 + j) * 2 * 128
                            nc.tensor.matmul(
                                z_ps,
                                w1t[:, base : base + 256].rearrange(
                                    "p (two m) -> p two m", two=2
                                ),
                                h2pair[j].rearrange("p (two n) -> p two n", two=2),
                                start=(j == 0),
                                stop=(j == 2),
                                perf_mode=DR,
                            )
                        jf, half = divmod(f, 2)
                        if half == 0:
                            zpair = apool.tile([128, 2 * T], F8, tag="zT", bufs=3)
                        nc.scalar.activation(
                            zpair[:, half * T : (half + 1) * T],
                            z_ps,
                            GELU,
                            bias=b1_sb[:, f : f + 1],
                            scale=1.0 / W_SC,
                        )
                        if half == 1:
                            jl = jf - cf * 3
                            for m in range(KT):
                                base = (jl * KT + m) * 2 * 128
                                nc.tensor.matmul(
                                    m_ps[m],
                                    w2t[:, base : base + 256].rearrange(
                                        "p (two m) -> p two m", two=2
                                    ),
                                    zpair.rearrange("p (two n) -> p two n", two=2),
                                    start=(jf == 0),
                                    stop=(jf == 2 * KT - 1),
                                    perf_mode=DR,
                                )
                    else:
                        for k in range(KT):
                            nc.tensor.matmul(
                                z_ps,
                                _mmv(w1t[:, (fi * KT + k) * 128 : (fi * KT + k + 1) * 128]),
                                _mmv(h2T[k]),
                                start=(k == 0),
                                stop=(k == KT - 1),
                            )
                        zt = apool.tile([128, T], dtw, tag="zT", bufs=4)
                        nc.scalar.activation(zt, z_ps, GELU, bias=b1_sb[:, f : f + 1])
                        for m in range(KT):
                            nc.tensor.matmul(
                                m_ps[m],
                                _mmv(w2t[:, (fi * KT + m) * 128 : (fi * KT + m + 1) * 128]),
                                _mmv(zt),
                                start=(f == 0),
                                stop=(f == 4 * KT - 1),
                            )
            nxt = []
            for m in range(KT):
                xn = apool.tile([128, T], xdt, tag="xT", bufs=8)
                if MLP_FP8:
                    t = apool.tile([128, T], xdt, tag="mres", bufs=2)
                    nc.vector.scalar_tensor_tensor(
                        t, m_ps[m], 1.0 / W_SC, x2T[m], op0=AO.mult, op1=AO.add
                    )
                    nc.vector.tensor_scalar(
                        xn, t, b2_sb[:, m : m + 1], None, op0=AO.add
                    )
                else:
                    nc.vector.scalar_tensor_tensor(
                        xn, m_ps[m], b2_sb[:, m : m + 1], x2T[m], op0=AO.add, op1=AO.add
                    )
                nxt.append(xn)
            return nxt

        for _rep in range(reps):
            # ---- x = emb + wpe, already in T-layout from the host ----
            xT = []
            for k in range(KT):
                xt = apool.tile([128, T], xdt, tag="xT", bufs=8)
                nc.vector.tensor_add(
                    xt,
                    xall[:, k * T : (k + 1) * T],
                    xall[:, KT * T + k * T : KT * T + (k + 1) * T],
                )
                xT.append(xt)

            for l in range(n_layers):
                xT = layer(l, xT)

            # ---- final LN (with gain/bias), stored in T-layout ----
            fout = apool.tile([128, KT * T], F32, tag="fout", bufs=1)
            layernorm(
                xT,
                F32,
                None,
                None,
                gcol=lnfg_sb,
                bcol=lnfb_sb,
                outs=[fout[:, k * T : (k + 1) * T] for k in range(KT)],
            )
            sdma(out_d, fout)

    nc.compile()
    return nc


class SpmdRunner:
    """Reusable jitted SPMD runner (modeled on bass2jax.run_bass_via_pjrt,
    without donation, so it can be invoked repeatedly)."""

    def __init__(self, nc, n_cores=N_CORES):
        bass2jax.install_neuronx_cc_hook()
        self.nc = nc
        self.n_cores = n_cores
        partition_name = nc.partition_id_tensor.name if nc.partition_id_tensor else None
        in_names, out_names, out_avals = [], [], []
        for alloc in nc.m.functions[0].allocations:
            if not isinstance(alloc, mybir.MemoryLocationSet):
                continue
            name = alloc.memorylocations[0].name
            if alloc.kind == "ExternalInput":
                if name != partition_name:
                    in_names.append(name)
            elif alloc.kind == "ExternalOutput":
                out_names.append(name)
                out_avals.append(
                    jax.core.ShapedArray(
                        tuple(alloc.tensor_shape), mybir.dt.np(alloc.dtype)
                    )
                )
        self.in_names, self.out_names, self.out_avals = in_names, out_names, out_avals
        n_params = len(in_names)
        all_in_names = list(in_names) + list(out_names)
        if partition_name is not None:
            all_in_names.append(partition_name)

        def _body(*args):
            operands = list(args)
            if partition_name is not None:
                operands.append(bass2jax.partition_id_tensor())
            outs = bass2jax._bass_exec_p.bind(
                *operands,
                out_avals=tuple(out_avals),
                in_names=tuple(all_in_names),
                out_names=tuple(out_names),
                lowering_input_output_aliases=(),
                sim_require_finite=True,
                sim_require_nnan=True,
                nc=nc,
            )
            return tuple(outs)

        devices = jax.devices()[:n_cores]
        self.mesh = Mesh(np.asarray(devices), ("core",))
        n_outs = len(out_names)
        in_specs = (PartitionSpec("core"),) * (n_params + n_outs)
        out_specs = (PartitionSpec("core"),) * n_outs
        self.fn = jax.jit(
            shard_map(
                _body,
                mesh=self.mesh,
                in_specs=in_specs,
                out_specs=out_specs,
                check_rep=False,
            ),
            keep_unused=True,
        )
        self.args = None

    def stage(self, in_maps):
        n = self.n_cores
        concat_in = [
            np.concatenate([np.asarray(in_maps[c][name]) for c in range(n)], axis=0)
            for name in self.in_names
        ]
        concat_zero = [
            np.zeros((n * a.shape[0], *a.shape[1:]), a.dtype) for a in self.out_avals
        ]
        sh = NamedSharding(self.mesh, PartitionSpec("core"))
        self.args = [jax.device_put(a, sh) for a in concat_in + concat_zero]

    def run(self):
        return self.fn(*self.args)

    def results(self, out_arrs):
        n = self.n_cores
        return [
            {
                name: np.asarray(out_arrs[i]).reshape(n, *self.out_avals[i].shape)[c]
                for i, name in enumerate(self.out_names)
            }
            for c in range(n)
        ]


def preprocess(inputs):
    """Host-side: fold LN gains into weights, shard tokens, build in_maps."""
    f = np.float32
    ie = np.asarray(inputs["inputs_embeds"], f)[0]  # [S, E]
    wpe = np.asarray(inputs["wpe"], f)[:S]
    g1 = np.asarray(inputs["ln1_g"], f)
    b1l = np.asarray(inputs["ln1_b"], f)
    g2 = np.asarray(inputs["ln2_g"], f)
    b2l = np.asarray(inputs["ln2_b"], f)
    Wq = np.asarray(inputs["Wq"], f)
    Wk = np.asarray(inputs["Wk"], f)
    Wv = np.asarray(inputs["Wv"], f)
    Wo = np.asarray(inputs["Wo"], f)
    W1 = np.asarray(inputs["W1"], f)
    W2 = np.asarray(inputs["W2"], f)
    bq = np.asarray(inputs["bq"], f)
    bk = np.asarray(inputs["bk"], f)
    bv = np.asarray(inputs["bv"], f)
    bo = np.asarray(inputs["bo"], f)
    b1 = np.asarray(inputs["b1"], f)
    b2 = np.asarray(inputs["b2"], f)

    scale = 1.0 / np.sqrt(DH)
    Wq_p = g1[:, :, None] * Wq * scale
    bq_p = (np.einsum("le,lef->lf", b1l, Wq) + bq) * scale
    Wk_p = g1[:, :, None] * Wk
    bk_p = np.einsum("le,lef->lf", b1l, Wk) + bk
    Wv_p = g1[:, :, None] * Wv
    bv_p = np.einsum("le,lef->lf", b1l, Wv) + bv
    Wkv = np.concatenate([Wk_p, Wv_p], axis=2)
    bkv = np.concatenate([bk_p, bv_p], axis=1)
    W1_p = g2[:, :, None] * W1
    b1_p = np.einsum("le,lef->lf", b2l, W1) + b1

    if COMPUTE == "bf16":
        cast = lambda a: np.ascontiguousarray(a).astype(ml_dtypes.bfloat16)
    else:
        cast = lambda a: np.ascontiguousarray(a, f)

    # prepack to [128, cols] SBUF images: col block k = rows k*128:(k+1)*128
    def pack2(a):  # [L, R, C] -> [L, 128, (R/128)*C]
        Lr, R, C = a.shape
        return (
            a.reshape(Lr, R // 128, 128, C)
            .transpose(0, 2, 1, 3)
            .reshape(Lr, 128, (R // 128) * C)
        )

    if MLP_FP8:
        f8 = ml_dtypes.float8_e4m3
        # chunk cf: block (fi, j) = two k-planes [k=2j | k=2j+1] of W1 cols f*128
        W1_pk = (
            (W1_p * W_SC)
            .reshape(L, 3, 2, 128, 4, KT, 128)
            .transpose(0, 4, 3, 5, 1, 2, 6)
            .reshape(L, 4, 128, KT * E)
            .astype(f8)
        )
        # chunk cf: block (jf, m) = two ff-planes [fi=2jf | fi=2jf+1] of W2 cols m*128
        W2_pk = (
            (W2 * W_SC)
            .reshape(L, 4, 3, 2, 128, KT, 128)
            .transpose(0, 1, 4, 2, 5, 3, 6)
            .reshape(L, 4, 128, KT * E)
            .astype(f8)
        )
    else:
        # chunk cf holds ff-tiles f=cf*6+fi; block (fi,k) at cols (fi*6+k)*128
        W1_pk = cast(
            W1_p.reshape(L, KT, 128, 4, KT, 128)
            .transpose(0, 3, 2, 4, 1, 5)
            .reshape(L, 4, 128, KT * E)
        )
        # chunk cf: block (fi,m) at cols (fi*6+m)*128 = W2[(cf*6+fi)*128.., m*128..]
        W2_pk = cast(
            W2.reshape(L, 4, KT, 128, KT, 128)
            .transpose(0, 1, 3, 2, 4, 5)
            .reshape(L, 4, 128, KT * E)
        )

    # packed bias const block [128, 180]
    def bpack(a, n):  # [L, n*128] -> [128, L*n]
        return a.reshape(L, n, 128).transpose(2, 0, 1).reshape(128, L * n)

    bias_blk = np.concatenate(
        [
            bpack(bq_p, KT),
            bpack(bo, KT),
            bpack(b2, KT),
            bpack(b1_p, FT),
            np.asarray(inputs["lnf_g"], f).reshape(KT, 128).T,
            np.asarray(inputs["lnf_b"], f).reshape(KT, 128).T,
        ],
        axis=1,
    )

    def tpack(a):  # [T, E] -> [128, KT*T] T-layout
        return a.reshape(T, KT, 128).transpose(2, 1, 0).reshape(128, KT * T)

    common = {
        "wq": cast(pack2(Wq_p)),
        "wkv": cast(pack2(Wkv)),
        "wo": cast(pack2(Wo)),
        "w1": np.ascontiguousarray(W1_pk) if MLP_FP8 else W1_pk,
        "w2": np.ascontiguousarray(W2_pk) if MLP_FP8 else W2_pk,
        "bias": np.ascontiguousarray(bias_blk, f),
        "bkv": cast(bkv),
    }
    maps = []
    for c in range(N_CORES):
        sl = slice(c * T, (c + 1) * T)
        xin = np.concatenate([tpack(ie[sl]), tpack(wpe[sl])], axis=1)
        maps.append({**common, "xin": np.ascontiguousarray(xin, f)})
    return maps


_RUNNER = None


def _get_runner():
    global _RUNNER
    if _RUNNER is None:
        nc = build_model(reps=1)
        _RUNNER = SpmdRunner(nc)
    return _RUNNER


def kernel(**inputs):
    runner = _get_runner()
    maps = preprocess(inputs)
    runner.stage(maps)
    outs = runner.run()
    res = runner.results(outs)
    full = np.concatenate(
        [
            res[c]["out"].reshape(128, KT, T).transpose(2, 1, 0).reshape(T, E)
            for c in range(N_CORES)
        ],
        axis=0,
    )
    return full[None].astype(np.float32)

